# revision 1
# baseline (speedup 1.0000x reference)
"""CLUB-NCE loss kernel for 8x Trainium2 NeuronCores (Bass/Tile).

Math (reference):
  hx = x @ W1x.T, hy = y @ W1y.T            [N, H]
  s[i,j]  = W2 . relu(hy[i] + hx[j] + b1) + b2
  T1[i,j] = softplus(s[i,j]); T0[i] = T1[i,i]
  lower = mean(T0) - (mean_i(logsumexp_j(T1[i,:])) - log N)
  upper = mean(T0) - mean(T1)

Sharding: y rows (i axis) split across 8 cores (64 rows each); x and MLP
params replicated. Each core computes its [64, 512] score block, converts
rows to exp-space (exp(softplus(s)) = 1 + e^s, so logsumexp over a row is
log(512 + sum_j e^s) with no max pass needed), and emits per-row partials
(row lse, row sum of T1, diag element). Host combines the scalar partials.

Device layout: contraction dim k (=H, padded 400->512) on partitions.
  hxT   [512k, 512j] fp16 (4 tiles of [128, 512])
  hybT  [512k,  64i] f32  (hy + b1, transposed)
  per i: r[kt] = fp16(relu(hxT[kt] + hybT[kt][:, i]))   (DVE 4x mode)
         psum[1, 512] += w2[kt].T @ r[kt]               (PE, fp16)
         E row = exp(psum + b2)                         (ACT, drains psum)
"""

import numpy as np

N = 512          # number of samples
D = 400          # feature dim
H = 400          # hidden dim
NCORES = 8
NL = N // NCORES  # 64 y-rows per core
KP = 512          # padded contraction dim
KT = 4            # 128-partition k tiles


def _build_program(b2val: float, enable_asserts: bool = False):
    import concourse.bacc as bacc
    import concourse.mybir as mybir
    import concourse.tile as tile

    fp16 = mybir.dt.float16
    f32 = mybir.dt.float32
    AF = mybir.ActivationFunctionType
    ALU = mybir.AluOpType

    nc = bacc.Bacc(
        "TRN2",
        target_bir_lowering=False,
        debug=False,
        enable_asserts=enable_asserts,
    )

    xT = nc.dram_tensor("xT", [KP, N], fp16, kind="ExternalInput")
    w1xT = nc.dram_tensor("w1xT", [KP, KP], fp16, kind="ExternalInput")
    w1yT = nc.dram_tensor("w1yT", [KP, KP], fp16, kind="ExternalInput")
    yT = nc.dram_tensor("yT", [KP, NL], fp16, kind="ExternalInput")
    b1c = nc.dram_tensor("b1c", [KP, 1], f32, kind="ExternalInput")
    w2c = nc.dram_tensor("w2c", [KP, 1], fp16, kind="ExternalInput")
    maskd = nc.dram_tensor("maskd", [NL, N], f32, kind="ExternalInput")

    lse_o = nc.dram_tensor("lse_o", [1, NL], f32, kind="ExternalOutput")
    rs_o = nc.dram_tensor("rs_o", [NL, 1], f32, kind="ExternalOutput")
    t0_o = nc.dram_tensor("t0_o", [NL, 1], f32, kind="ExternalOutput")

    eflat_d = nc.dram_tensor("eflat_d", [1, NL * N], f32)  # bounce buffer

    with tile.TileContext(nc) as tc:
        with (
            tc.tile_pool(name="const", bufs=1) as cpool,
            tc.tile_pool(name="work", bufs=32) as wpool,
            tc.tile_pool(name="ppro", bufs=2, space="PSUM") as ppro,
            tc.tile_pool(name="pmain", bufs=6, space="PSUM") as pmain,
        ):
            xt, w1x, w1y, yt, b1t, w2t = [], [], [], [], [], []
            for k in range(KT):
                sl = slice(k * 128, (k + 1) * 128)
                t = cpool.tile([128, N], fp16, name=f"xt{k}")
                nc.sync.dma_start(out=t, in_=xT[sl, :])
                xt.append(t)
                t = cpool.tile([128, KP], fp16, name=f"w1x{k}")
                nc.sync.dma_start(out=t, in_=w1xT[sl, :])
                w1x.append(t)
                t = cpool.tile([128, KP], fp16, name=f"w1y{k}")
                nc.sync.dma_start(out=t, in_=w1yT[sl, :])
                w1y.append(t)
                t = cpool.tile([128, NL], fp16, name=f"yt{k}")
                nc.sync.dma_start(out=t, in_=yT[sl, :])
                yt.append(t)
                t = cpool.tile([128, 1], f32, name=f"b1t{k}")
                nc.sync.dma_start(out=t, in_=b1c[sl, :])
                b1t.append(t)
                t = cpool.tile([128, 1], fp16, name=f"w2t{k}")
                nc.sync.dma_start(out=t, in_=w2c[sl, :])
                w2t.append(t)
            mask = cpool.tile([NL, N], f32, name="mask")
            nc.sync.dma_start(out=mask, in_=maskd[:, :])
            b2t = cpool.tile([1, 1], f32, name="b2t")
            nc.vector.memset(b2t, b2val)
            n512t = cpool.tile([1, 1], f32, name="n512t")
            nc.vector.memset(n512t, float(N))

            # ---- prologue: hxT (fp16) and hybT (f32) ----
            hx, hyb = [], []
            for m in range(KT):
                msl = slice(m * 128, (m + 1) * 128)
                ph = ppro.tile([128, N], f32, name=f"ph{m}", tag="pp")
                for k in range(KT):
                    nc.tensor.matmul(
                        ph, lhsT=w1x[k][:, msl], rhs=xt[k],
                        start=(k == 0), stop=(k == KT - 1),
                    )
                hxm = cpool.tile([128, N], fp16, name=f"hx{m}")
                nc.vector.tensor_copy(out=hxm, in_=ph)
                hx.append(hxm)
            for m in range(KT):
                msl = slice(m * 128, (m + 1) * 128)
                py = ppro.tile([128, NL], f32, name=f"py{m}", tag="pp")
                for k in range(KT):
                    nc.tensor.matmul(
                        py, lhsT=w1y[k][:, msl], rhs=yt[k],
                        start=(k == 0), stop=(k == KT - 1),
                    )
                hybm = cpool.tile([128, NL], f32, name=f"hyb{m}")
                nc.vector.tensor_scalar_add(hybm, py, b1t[m])
                hyb.append(hybm)

            # ---- main loop over local y rows ----
            eflat = cpool.tile([1, NL * N], f32, name="eflat")
            rrow = cpool.tile([1, NL], f32, name="rrow")
            for i in range(NL):
                ps = pmain.tile([1, N], f32, name="ps", tag="ps")
                for k in range(KT):
                    r = wpool.tile([128, N], fp16, name="r", tag="r")
                    nc.vector.tensor_scalar(
                        out=r, in0=hx[k],
                        scalar1=hyb[k][:, i : i + 1], scalar2=0.0,
                        op0=ALU.add, op1=ALU.max,
                    )
                    nc.tensor.matmul(
                        ps, lhsT=w2t[k], rhs=r,
                        start=(k == 0), stop=(k == KT - 1),
                    )
                # drain psum row: E = exp(s + b2), R[i] = sum_j E
                nc.scalar.activation(
                    out=eflat[:, i * N : (i + 1) * N], in_=ps,
                    func=AF.Exp, bias=b2t[0:1, :], scale=1.0,
                    accum_out=rrow[:, i : i + 1],
                )

            # ---- restructure E rows [1, NL*N] -> [NL, N] via DRAM bounce ----
            nc.sync.dma_start(out=eflat_d[:, :], in_=eflat)
            e2 = cpool.tile([NL, N], f32, name="e2")
            nc.sync.dma_start(
                out=e2, in_=eflat_d.ap().rearrange("o (i j) -> (o i) j", i=NL)
            )

            # ---- postprocessing ----
            t1 = cpool.tile([NL, N], f32, name="t1")
            rs = cpool.tile([NL, 1], f32, name="rs")
            # T1 = log(1 + E) = softplus(s); rs = row sums of T1
            nc.scalar.activation(
                out=t1, in_=e2, func=AF.Ln, bias=1.0, scale=1.0
            )
            nc.vector.reduce_sum(out=rs, in_=t1, axis=mybir.AxisListType.X)
            lse = cpool.tile([1, NL], f32, name="lse")
            # row logsumexp = log(512 + sum_j e^s)
            nc.scalar.activation(
                out=lse, in_=rrow, func=AF.Ln, bias=n512t[0:1, :], scale=1.0
            )
            junk = cpool.tile([NL, N], f32, name="junk")
            t0 = cpool.tile([NL, 1], f32, name="t0")
            nc.vector.tensor_tensor(
                out=junk, in0=t1, in1=mask, op=ALU.mult
            )
            nc.vector.reduce_sum(out=t0, in_=junk, axis=mybir.AxisListType.X)
            nc.sync.dma_start(out=lse_o[:, :], in_=lse)
            nc.sync.dma_start(out=rs_o[:, :], in_=rs)
            nc.sync.dma_start(out=t0_o[:, :], in_=t0)

    nc.compile()
    return nc


def _make_in_maps(x, y, W1, b1, W2):
    f16 = np.float16
    xTp = np.zeros((KP, N), f16)
    xTp[:D, :] = x.T.astype(f16)
    w1xTp = np.zeros((KP, KP), f16)
    w1xTp[:D, :H] = W1[:, :D].T.astype(f16)
    w1yTp = np.zeros((KP, KP), f16)
    w1yTp[:D, :H] = W1[:, D:].T.astype(f16)
    b1p = np.zeros((KP, 1), np.float32)
    b1p[:H, 0] = b1
    w2p = np.zeros((KP, 1), f16)
    w2p[:H, 0] = W2[0].astype(f16)

    in_maps = []
    for c in range(NCORES):
        yTp = np.zeros((KP, NL), f16)
        yTp[:D, :] = y[c * NL : (c + 1) * NL, :].T.astype(f16)
        mask = np.zeros((NL, N), np.float32)
        mask[np.arange(NL), c * NL + np.arange(NL)] = 1.0
        in_maps.append(
            {
                "xT": xTp, "w1xT": w1xTp, "w1yT": w1yTp, "yT": yTp,
                "b1c": b1p, "w2c": w2p, "maskd": mask,
            }
        )
    return in_maps


def _combine(results):
    lse_all = np.concatenate([r["lse_o"][0].astype(np.float64) for r in results])
    rs_all = np.concatenate([r["rs_o"][:, 0].astype(np.float64) for r in results])
    t0_all = np.concatenate([r["t0_o"][:, 0].astype(np.float64) for r in results])
    t0_mean = t0_all.mean()
    lower = t0_mean - (lse_all.mean() - np.log(np.float64(N)))
    upper = t0_mean - rs_all.mean() / N
    return np.float32(lower), np.float32(upper)


def kernel(x_samples, y_samples, W1, b1, W2, b2, _trace=False):
    from concourse.bass_utils import run_bass_kernel_spmd

    nc = _build_program(float(np.float32(b2[0])))
    in_maps = _make_in_maps(
        np.asarray(x_samples, np.float32),
        np.asarray(y_samples, np.float32),
        np.asarray(W1, np.float32),
        np.asarray(b1, np.float32),
        np.asarray(W2, np.float32),
    )
    res = run_bass_kernel_spmd(
        nc, in_maps, core_ids=list(range(NCORES)), trace=_trace
    )
    out = _combine(res.results)
    if _trace:
        return out, res
    return out



# revision 7
# speedup vs baseline: 1.2291x; 1.2291x over previous
"""CLUB-NCE loss kernel for 8x Trainium2 NeuronCores (Bass/Tile). v2

Math (reference):
  hx = x @ W1x.T, hy = y @ W1y.T            [N, H]
  s[i,j]  = W2 . relu(hy[i] + hx[j] + b1) + b2
  T1[i,j] = softplus(s[i,j]); T0[i] = T1[i,i]
  lower = mean(T0) - (mean_i(logsumexp_j(T1[i,:])) - log N)
  upper = mean(T0) - mean(T1)

Sharding: y rows (i axis) split across 8 cores (64 rows each); x and MLP
params replicated. Each core computes its [64, 512] score block in ONE
psum bank: row i of the block accumulates at psum partition sigma(i),
so the entire epilogue (exp, softplus, row sums, diag extract) runs as a
handful of [128, 512] batched ops instead of 64 per-row ops.

i -> partition map: i = 8u + o (u,o in 0..7); col-group g = o % 4,
phat = o // 4, slot s = 8*phat + u; partition = 32g + s. Each (i, k<3)
matmul uses lhsT = [128, 32] slice of a [128, 48] window tensor that
places w2[k-tile] in output column s; out = PS[32g:32g+32, :] so the
4 col-groups can run concurrently on hardware (tile_position derives
from out.base_partition). The h-tail (h 384..399, 16 rows) is packed
8-i-per-op: partitions (16u + h) with a block-diagonal lhsT.

relu generation is split DVE (tensor_scalar, 4x mode) / ACT (activation
Relu with per-partition bias) to balance engine time.
"""

import numpy as np

N = 512          # number of samples
D = 400          # feature dim
H = 400          # hidden dim
NCORES = 8
NL = N // NCORES  # 64 y-rows per core
KP = 512          # padded contraction dim (d)
KT = 4            # 128-partition d tiles
HT = 3            # full 128-row h tiles (h 0..383); tail h 384..399 packed

ACT_S = (3, 7, 11, 15)  # slots whose (i, k<3) relu ops run on ACT engine


def _sigma(i):
    """i -> psum partition."""
    u, o = i // 8, i % 8
    g, phat = o % 4, o // 4
    return 32 * g + 8 * phat + u


def _build_program(b2val: float, enable_asserts: bool = False):
    import concourse.bacc as bacc
    import concourse.mybir as mybir
    import concourse.tile as tile

    fp16 = mybir.dt.float16
    f32 = mybir.dt.float32
    AF = mybir.ActivationFunctionType
    ALU = mybir.AluOpType

    nc = bacc.Bacc(
        "TRN2",
        target_bir_lowering=False,
        debug=False,
        enable_asserts=enable_asserts,
    )

    xT = nc.dram_tensor("xT", [KP, N], fp16, kind="ExternalInput")
    w1xT = nc.dram_tensor("w1xT", [KP, KP], fp16, kind="ExternalInput")
    w1yT = nc.dram_tensor("w1yT", [KP, KP], fp16, kind="ExternalInput")
    w1x3r = nc.dram_tensor("w1x3r", [KP, 128], fp16, kind="ExternalInput")
    yT = nc.dram_tensor("yT", [KP, NL], fp16, kind="ExternalInput")
    b1c = nc.dram_tensor("b1c", [KP, 1], f32, kind="ExternalInput")
    w2win = nc.dram_tensor("w2win", [HT * 128, 48], fp16, kind="ExternalInput")
    w23bd = nc.dram_tensor("w23bd", [128, 64], fp16, kind="ExternalInput")
    maskd = nc.dram_tensor("maskd", [128, N], f32, kind="ExternalInput")

    lse_o = nc.dram_tensor("lse_o", [128, 1], f32, kind="ExternalOutput")
    rs_o = nc.dram_tensor("rs_o", [128, 1], f32, kind="ExternalOutput")
    t0_o = nc.dram_tensor("t0_o", [128, 1], f32, kind="ExternalOutput")

    hyb3_d = nc.dram_tensor("hyb3_d", [16, NL], f32)  # bounce for h-tail gather

    with tile.TileContext(nc) as tc:
        with (
            tc.tile_pool(name="const", bufs=1) as cpool,
            tc.tile_pool(name="work", bufs=24) as wpool,
            tc.tile_pool(name="ppro", bufs=2, space="PSUM") as ppro,
            tc.tile_pool(name="pmain", bufs=1, space="PSUM") as pmain,
        ):
            # ---- constant loads ----
            xt, w1x, w1y, w1x3, yt, b1t = [], [], [], [], [], []
            for k in range(KT):
                sl = slice(k * 128, (k + 1) * 128)
                t = cpool.tile([128, N], fp16, name=f"xt{k}")
                nc.sync.dma_start(out=t, in_=xT[sl, :])
                xt.append(t)
                t = cpool.tile([128, KP], fp16, name=f"w1x{k}")
                nc.sync.dma_start(out=t, in_=w1xT[sl, :])
                w1x.append(t)
                t = cpool.tile([128, KP], fp16, name=f"w1y{k}")
                nc.sync.dma_start(out=t, in_=w1yT[sl, :])
                w1y.append(t)
                t = cpool.tile([128, 128], fp16, name=f"w1x3_{k}")
                nc.sync.dma_start(out=t, in_=w1x3r[sl, :])
                w1x3.append(t)
                t = cpool.tile([128, NL], fp16, name=f"yt{k}")
                nc.sync.dma_start(out=t, in_=yT[sl, :])
                yt.append(t)
                t = cpool.tile([128, 1], f32, name=f"b1t{k}")
                nc.sync.dma_start(out=t, in_=b1c[sl, :])
                b1t.append(t)
            w2w = []
            for k in range(HT):
                t = cpool.tile([128, 48], fp16, name=f"w2w{k}")
                nc.sync.dma_start(out=t, in_=w2win[k * 128 : (k + 1) * 128, :])
                w2w.append(t)
            w23 = cpool.tile([128, 64], fp16, name="w23")
            nc.sync.dma_start(out=w23, in_=w23bd[:, :])
            mask = cpool.tile([128, N], f32, name="mask")
            nc.sync.dma_start(out=mask, in_=maskd[:, :])
            b2t = cpool.tile([128, 1], f32, name="b2t")
            nc.vector.memset(b2t, b2val)
            onet = cpool.tile([128, 1], f32, name="onet")
            nc.vector.memset(onet, 1.0)
            n512t = cpool.tile([128, 1], f32, name="n512t")
            nc.vector.memset(n512t, float(N))

            # ---- prologue: hy first (small MMs), then hx ----
            # hyb[m] = (y @ W1y.T).T tile m + b1   [128, NL] f32, m=0..2
            hyb = []
            for m in range(HT):
                msl = slice(m * 128, (m + 1) * 128)
                py = ppro.tile([128, NL], f32, name=f"py{m}", tag="pp")
                for k in range(KT):
                    nc.tensor.matmul(
                        py, lhsT=w1y[k][:, msl], rhs=yt[k],
                        start=(k == 0), stop=(k == KT - 1),
                    )
                hybm = cpool.tile([128, NL], f32, name=f"hyb{m}")
                nc.vector.tensor_scalar_add(hybm, py, b1t[m])
                hyb.append(hybm)
            # h-tail: hyb3 [16, NL] -> DRAM bounce -> hyb3p [128, 8]
            # hyb3p[16u + h, o] = hyb3[h, 8u + o]
            py3 = ppro.tile([16, NL], f32, name="py3", tag="pp")
            for k in range(KT):
                nc.tensor.matmul(
                    py3, lhsT=w1y[k][:, 384:400], rhs=yt[k],
                    start=(k == 0), stop=(k == KT - 1),
                )
            hyb3 = cpool.tile([16, NL], f32, name="hyb3")
            nc.vector.tensor_scalar_add(hyb3, py3, b1t[3][0:16, :])
            nc.sync.dma_start(out=hyb3_d[:, :], in_=hyb3)
            hyb3p = cpool.tile([128, 8], f32, name="hyb3p")
            for u in range(8):
                nc.sync.dma_start(
                    out=hyb3p[16 * u : 16 * u + 16, :],
                    in_=hyb3_d[:, 8 * u : 8 * u + 8],
                )

            # hx[m] = (x @ W1x.T).T tile m  [128, N] fp16, m=0..2
            hx = []
            for m in range(HT):
                msl = slice(m * 128, (m + 1) * 128)
                ph = ppro.tile([128, N], f32, name=f"ph{m}", tag="pp")
                for k in range(KT):
                    nc.tensor.matmul(
                        ph, lhsT=w1x[k][:, msl], rhs=xt[k],
                        start=(k == 0), stop=(k == KT - 1),
                    )
                hxm = cpool.tile([128, N], fp16, name=f"hx{m}")
                nc.vector.tensor_copy(out=hxm, in_=ph)
                hx.append(hxm)
            # h-tail replicated: hx3rep[16u + h, j] = hx[384 + h, j]
            ph3 = ppro.tile([128, N], f32, name="ph3", tag="pp")
            for k in range(KT):
                nc.tensor.matmul(
                    ph3, lhsT=w1x3[k], rhs=xt[k],
                    start=(k == 0), stop=(k == KT - 1),
                )
            hx3rep = cpool.tile([128, N], fp16, name="hx3rep")
            nc.vector.tensor_copy(out=hx3rep, in_=ph3)

            # ---- main: accumulate all 64 score rows into one psum bank ----
            ps = pmain.tile([128, N], f32, name="ps")

            def gen_r(i, k):
                r = wpool.tile([128, N], fp16, name="r", tag="r")
                if (_sigma(i) % 32) in ACT_S:
                    nc.scalar.activation(
                        out=r, in_=hx[k], func=AF.Relu,
                        bias=hyb[k][:, i : i + 1], scale=1.0,
                    )
                else:
                    nc.vector.tensor_scalar(
                        out=r, in0=hx[k],
                        scalar1=hyb[k][:, i : i + 1], scalar2=0.0,
                        op0=ALU.add, op1=ALU.max,
                    )
                return r

            for k in range(HT):
                for s in range(16):
                    for g in range(4):
                        u, phat = s % 8, s // 8
                        i = 8 * u + g + 4 * phat
                        r = gen_r(i, k)
                        nc.tensor.matmul(
                            ps[32 * g : 32 * g + 32, :],
                            lhsT=w2w[k][:, 15 - s : 47 - s],
                            rhs=r,
                            start=(k == 0 and s == 0),
                            stop=False,
                            skip_group_check=True,
                            tile_position=(0, 32 * g),
                        )
            # h-tail: one packed relu + block-diag matmul per 8 rows
            for phat in range(2):
                for g in range(4):
                    o = g + 4 * phat
                    r3 = wpool.tile([128, N], fp16, name="r3", tag="r")
                    nc.vector.tensor_scalar(
                        out=r3, in0=hx3rep,
                        scalar1=hyb3p[:, o : o + 1], scalar2=0.0,
                        op0=ALU.add, op1=ALU.max,
                    )
                    nc.tensor.matmul(
                        ps[32 * g : 32 * g + 32, :],
                        lhsT=w23[:, 32 * phat : 32 * phat + 32],
                        rhs=r3,
                        start=False,
                        stop=(phat == 1),
                        skip_group_check=True,
                        tile_position=(0, 32 * g),
                    )

            # ---- epilogue: batched over all 64 rows at once ----
            ecols = cpool.tile([128, N], f32, name="ecols")
            rsum_e = cpool.tile([128, 1], f32, name="rsum_e")
            # E = exp(s + b2); row sums of E via accum
            nc.scalar.activation(
                out=ecols, in_=ps, func=AF.Exp, bias=b2t, scale=1.0,
                accum_out=rsum_e,
            )
            t1 = cpool.tile([128, N], f32, name="t1")
            nc.scalar.activation(out=t1, in_=ecols, func=AF.Ln, bias=onet, scale=1.0)
            rs = cpool.tile([128, 1], f32, name="rs")
            nc.vector.reduce_sum(out=rs, in_=t1, axis=mybir.AxisListType.X)
            junk = cpool.tile([128, N], f32, name="junk")
            nc.vector.tensor_tensor(out=junk, in0=t1, in1=mask, op=ALU.mult)
            t0 = cpool.tile([128, 1], f32, name="t0")
            nc.vector.reduce_sum(out=t0, in_=junk, axis=mybir.AxisListType.X)
            lse = cpool.tile([128, 1], f32, name="lse")
            # row logsumexp = log(512 + sum_j e^s)
            nc.scalar.activation(
                out=lse, in_=rsum_e, func=AF.Ln, bias=n512t, scale=1.0
            )
            nc.sync.dma_start(out=lse_o[:, :], in_=lse)
            nc.sync.dma_start(out=rs_o[:, :], in_=rs)
            nc.sync.dma_start(out=t0_o[:, :], in_=t0)

    nc.compile()
    return nc


def _make_in_maps(x, y, W1, b1, W2):
    f16 = np.float16
    xTp = np.zeros((KP, N), f16)
    xTp[:D, :] = x.T.astype(f16)
    w1xTp = np.zeros((KP, KP), f16)
    w1xTp[:D, :H] = W1[:, :D].T.astype(f16)
    w1yTp = np.zeros((KP, KP), f16)
    w1yTp[:D, :H] = W1[:, D:].T.astype(f16)
    w1x3rp = np.tile(w1xTp[:, 384:400], (1, 8)).copy()
    b1p = np.zeros((KP, 1), np.float32)
    b1p[:H, 0] = b1
    # w2 slot-window: w2win[k*128 + h, 15] = W2[0, k*128 + h]
    w2winp = np.zeros((HT * 128, 48), f16)
    w2winp[:, 15] = W2[0, : HT * 128].astype(f16)
    # h-tail block-diagonal: col (32*phat + 8*phat + u) gets w2[384 + h]
    # at partition 16u + h
    w23bdp = np.zeros((128, 64), f16)
    for phat in range(2):
        for u in range(8):
            for h in range(16):
                w23bdp[16 * u + h, 32 * phat + 8 * phat + u] = np.float16(
                    W2[0, 384 + h]
                )

    in_maps = []
    for c in range(NCORES):
        yTp = np.zeros((KP, NL), f16)
        yTp[:D, :] = y[c * NL : (c + 1) * NL, :].T.astype(f16)
        mask = np.zeros((128, N), np.float32)
        for i in range(NL):
            mask[_sigma(i), c * NL + i] = 1.0
        in_maps.append(
            {
                "xT": xTp, "w1xT": w1xTp, "w1yT": w1yTp, "w1x3r": w1x3rp,
                "yT": yTp, "b1c": b1p, "w2win": w2winp, "w23bd": w23bdp,
                "maskd": mask,
            }
        )
    return in_maps


def _combine(results):
    perm = np.array([_sigma(i) for i in range(NL)])
    lse_all = np.concatenate(
        [r["lse_o"][perm, 0].astype(np.float64) for r in results]
    )
    rs_all = np.concatenate(
        [r["rs_o"][perm, 0].astype(np.float64) for r in results]
    )
    t0_all = np.concatenate(
        [r["t0_o"][perm, 0].astype(np.float64) for r in results]
    )
    t0_mean = t0_all.mean()
    lower = t0_mean - (lse_all.mean() - np.log(np.float64(N)))
    upper = t0_mean - rs_all.mean() / N
    return np.float32(lower), np.float32(upper)


def kernel(x_samples, y_samples, W1, b1, W2, b2, _trace=False):
    from concourse.bass_utils import run_bass_kernel_spmd

    nc = _build_program(float(np.float32(b2[0])))
    in_maps = _make_in_maps(
        np.asarray(x_samples, np.float32),
        np.asarray(y_samples, np.float32),
        np.asarray(W1, np.float32),
        np.asarray(b1, np.float32),
        np.asarray(W2, np.float32),
    )
    res = run_bass_kernel_spmd(
        nc, in_maps, core_ids=list(range(NCORES)), trace=_trace
    )
    out = _combine(res.results)
    if _trace:
        return out, res
    return out


# revision 9
# speedup vs baseline: 1.2327x; 1.0029x over previous
"""CLUB-NCE loss kernel for 8x Trainium2 NeuronCores (Bass/Tile). v2

Math (reference):
  hx = x @ W1x.T, hy = y @ W1y.T            [N, H]
  s[i,j]  = W2 . relu(hy[i] + hx[j] + b1) + b2
  T1[i,j] = softplus(s[i,j]); T0[i] = T1[i,i]
  lower = mean(T0) - (mean_i(logsumexp_j(T1[i,:])) - log N)
  upper = mean(T0) - mean(T1)

Sharding: y rows (i axis) split across 8 cores (64 rows each); x and MLP
params replicated. Each core computes its [64, 512] score block in ONE
psum bank: row i of the block accumulates at psum partition sigma(i),
so the entire epilogue (exp, softplus, row sums, diag extract) runs as a
handful of [128, 512] batched ops instead of 64 per-row ops.

i -> partition map: i = 8u + o (u,o in 0..7); col-group g = o % 4,
phat = o // 4, slot s = 8*phat + u; partition = 32g + s. Each (i, k<3)
matmul uses lhsT = [128, 32] slice of a [128, 48] window tensor that
places w2[k-tile] in output column s; out = PS[32g:32g+32, :] so the
4 col-groups can run concurrently on hardware (tile_position derives
from out.base_partition). The h-tail (h 384..399, 16 rows) is packed
8-i-per-op: partitions (16u + h) with a block-diagonal lhsT.

relu generation is split DVE (tensor_scalar, 4x mode) / ACT (activation
Relu with per-partition bias) to balance engine time.
"""

import numpy as np

N = 512          # number of samples
D = 400          # feature dim
H = 400          # hidden dim
NCORES = 8
NL = N // NCORES  # 64 y-rows per core
KP = 512          # padded contraction dim (d)
KT = 4            # 128-partition d tiles
HT = 3            # full 128-row h tiles (h 0..383); tail h 384..399 packed

ACT_S = (3, 7, 11, 15)  # slots whose (i, k<3) relu ops run on ACT engine


def _sigma(i):
    """i -> psum partition."""
    u, o = i // 8, i % 8
    g, phat = o % 4, o // 4
    return 32 * g + 8 * phat + u


def _build_program(b2val: float, enable_asserts: bool = False):
    import concourse.bacc as bacc
    import concourse.mybir as mybir
    import concourse.tile as tile

    fp16 = mybir.dt.float16
    f32 = mybir.dt.float32
    AF = mybir.ActivationFunctionType
    ALU = mybir.AluOpType

    nc = bacc.Bacc(
        "TRN2",
        target_bir_lowering=False,
        debug=False,
        enable_asserts=enable_asserts,
    )

    xT = nc.dram_tensor("xT", [KP, N], fp16, kind="ExternalInput")
    w1xT = nc.dram_tensor("w1xT", [KP, KP], fp16, kind="ExternalInput")
    w1yT = nc.dram_tensor("w1yT", [KP, KP], fp16, kind="ExternalInput")
    w1x3r = nc.dram_tensor("w1x3r", [KP, 128], fp16, kind="ExternalInput")
    yT = nc.dram_tensor("yT", [KP, NL], fp16, kind="ExternalInput")
    b1c = nc.dram_tensor("b1c", [KP, 1], f32, kind="ExternalInput")
    w2win = nc.dram_tensor("w2win", [HT * 128, 48], fp16, kind="ExternalInput")
    w23bd = nc.dram_tensor("w23bd", [128, 64], fp16, kind="ExternalInput")
    maskd = nc.dram_tensor("maskd", [128, N], f32, kind="ExternalInput")

    lse_o = nc.dram_tensor("lse_o", [128, 1], f32, kind="ExternalOutput")
    rs_o = nc.dram_tensor("rs_o", [128, 1], f32, kind="ExternalOutput")
    t0_o = nc.dram_tensor("t0_o", [128, 1], f32, kind="ExternalOutput")

    hyb3_d = nc.dram_tensor("hyb3_d", [16, NL], f32)  # bounce for h-tail gather

    with tile.TileContext(nc) as tc:
        with (
            tc.tile_pool(name="const", bufs=1) as cpool,
            tc.tile_pool(name="work", bufs=24) as wpool,
            tc.tile_pool(name="ppro", bufs=2, space="PSUM") as ppro,
            tc.tile_pool(name="pmain", bufs=1, space="PSUM") as pmain,
        ):
            # ---- constant loads ----
            xt, w1x, w1y, w1x3, yt, b1t = [], [], [], [], [], []
            for k in range(KT):
                sl = slice(k * 128, (k + 1) * 128)
                t = cpool.tile([128, N], fp16, name=f"xt{k}")
                nc.sync.dma_start(out=t, in_=xT[sl, :])
                xt.append(t)
                t = cpool.tile([128, KP], fp16, name=f"w1x{k}")
                nc.sync.dma_start(out=t, in_=w1xT[sl, :])
                w1x.append(t)
                t = cpool.tile([128, KP], fp16, name=f"w1y{k}")
                nc.sync.dma_start(out=t, in_=w1yT[sl, :])
                w1y.append(t)
                t = cpool.tile([128, 128], fp16, name=f"w1x3_{k}")
                nc.sync.dma_start(out=t, in_=w1x3r[sl, :])
                w1x3.append(t)
                t = cpool.tile([128, NL], fp16, name=f"yt{k}")
                nc.sync.dma_start(out=t, in_=yT[sl, :])
                yt.append(t)
                t = cpool.tile([128, 1], f32, name=f"b1t{k}")
                nc.sync.dma_start(out=t, in_=b1c[sl, :])
                b1t.append(t)
            w2w = []
            for k in range(HT):
                t = cpool.tile([128, 48], fp16, name=f"w2w{k}")
                nc.sync.dma_start(out=t, in_=w2win[k * 128 : (k + 1) * 128, :])
                w2w.append(t)
            w23 = cpool.tile([128, 64], fp16, name="w23")
            nc.sync.dma_start(out=w23, in_=w23bd[:, :])
            mask = cpool.tile([128, N], f32, name="mask")
            nc.sync.dma_start(out=mask, in_=maskd[:, :])
            b2t = cpool.tile([128, 1], f32, name="b2t")
            nc.vector.memset(b2t, b2val)
            onet = cpool.tile([128, 1], f32, name="onet")
            nc.vector.memset(onet, 1.0)
            n512t = cpool.tile([128, 1], f32, name="n512t")
            nc.vector.memset(n512t, float(N))

            # ---- prologue: hy first (small MMs), then hx ----
            # hyb[m] = (y @ W1y.T).T tile m + b1   [128, NL] f32, m=0..2
            hyb = []
            for m in range(HT):
                msl = slice(m * 128, (m + 1) * 128)
                py = ppro.tile([128, NL], f32, name=f"py{m}", tag="pp")
                for k in range(KT):
                    nc.tensor.matmul(
                        py, lhsT=w1y[k][:, msl], rhs=yt[k],
                        start=(k == 0), stop=(k == KT - 1),
                    )
                hybm = cpool.tile([128, NL], f32, name=f"hyb{m}")
                nc.vector.tensor_scalar_add(hybm, py, b1t[m])
                hyb.append(hybm)
            # h-tail: hyb3 [16, NL] -> DRAM bounce -> hyb3p [128, 8]
            # hyb3p[16u + h, o] = hyb3[h, 8u + o]
            py3 = ppro.tile([16, NL], f32, name="py3", tag="pp")
            for k in range(KT):
                nc.tensor.matmul(
                    py3, lhsT=w1y[k][:, 384:400], rhs=yt[k],
                    start=(k == 0), stop=(k == KT - 1),
                )
            hyb3 = cpool.tile([16, NL], f32, name="hyb3")
            nc.vector.tensor_scalar_add(hyb3, py3, b1t[3][0:16, :])
            nc.sync.dma_start(out=hyb3_d[:, :], in_=hyb3)
            hyb3p = cpool.tile([128, 8], f32, name="hyb3p")
            for u in range(8):
                nc.sync.dma_start(
                    out=hyb3p[16 * u : 16 * u + 16, :],
                    in_=hyb3_d[:, 8 * u : 8 * u + 8],
                )

            # hx[m] = (x @ W1x.T).T tile m  [128, N] fp16, m=0..2
            hx = []
            for m in range(HT):
                msl = slice(m * 128, (m + 1) * 128)
                ph = ppro.tile([128, N], f32, name=f"ph{m}", tag="pp")
                for k in range(KT):
                    nc.tensor.matmul(
                        ph, lhsT=w1x[k][:, msl], rhs=xt[k],
                        start=(k == 0), stop=(k == KT - 1),
                    )
                hxm = cpool.tile([128, N], fp16, name=f"hx{m}")
                nc.vector.tensor_copy(out=hxm, in_=ph)
                hx.append(hxm)
            # h-tail replicated: hx3rep[16u + h, j] = hx[384 + h, j]
            ph3 = ppro.tile([128, N], f32, name="ph3", tag="pp")
            for k in range(KT):
                nc.tensor.matmul(
                    ph3, lhsT=w1x3[k], rhs=xt[k],
                    start=(k == 0), stop=(k == KT - 1),
                )
            hx3rep = cpool.tile([128, N], fp16, name="hx3rep")
            nc.vector.tensor_copy(out=hx3rep, in_=ph3)

            # ---- main: accumulate all 64 score rows into one psum bank ----
            ps = pmain.tile([128, N], f32, name="ps")

            def gen_r(i, k, on_act):
                r = wpool.tile([128, N], fp16, name="r", tag="r")
                if on_act:
                    nc.scalar.activation(
                        out=r, in_=hx[k], func=AF.Relu,
                        bias=hyb[k][:, i : i + 1], scale=1.0,
                    )
                else:
                    nc.vector.tensor_scalar(
                        out=r, in0=hx[k],
                        scalar1=hyb[k][:, i : i + 1], scalar2=0.0,
                        op0=ALU.add, op1=ALU.max,
                    )
                return r

            for k in range(HT):
                for s in range(16):
                    for g in range(4):
                        u, phat = s % 8, s // 8
                        i = 8 * u + g + 4 * phat
                        r = gen_r(i, k, on_act=(g == 3))
                        nc.tensor.matmul(
                            ps[32 * g : 32 * g + 32, :],
                            lhsT=w2w[k][:, 15 - s : 47 - s],
                            rhs=r,
                            start=(k == 0 and s == 0),
                            stop=False,
                            skip_group_check=True,
                            tile_position=(0, 32 * g),
                        )
            # h-tail: one packed relu + block-diag matmul per 8 rows
            for phat in range(2):
                for g in range(4):
                    o = g + 4 * phat
                    r3 = wpool.tile([128, N], fp16, name="r3", tag="r")
                    nc.vector.tensor_scalar(
                        out=r3, in0=hx3rep,
                        scalar1=hyb3p[:, o : o + 1], scalar2=0.0,
                        op0=ALU.add, op1=ALU.max,
                    )
                    nc.tensor.matmul(
                        ps[32 * g : 32 * g + 32, :],
                        lhsT=w23[:, 32 * phat : 32 * phat + 32],
                        rhs=r3,
                        start=False,
                        stop=(phat == 1),
                        skip_group_check=True,
                        tile_position=(0, 32 * g),
                    )

            # ---- epilogue: batched over all 64 rows at once ----
            ecols = cpool.tile([128, N], f32, name="ecols")
            rsum_e = cpool.tile([128, 1], f32, name="rsum_e")
            # E = exp(s + b2); row sums of E via accum
            nc.scalar.activation(
                out=ecols, in_=ps, func=AF.Exp, bias=b2t, scale=1.0,
                accum_out=rsum_e,
            )
            t1 = cpool.tile([128, N], f32, name="t1")
            nc.scalar.activation(out=t1, in_=ecols, func=AF.Ln, bias=onet, scale=1.0)
            rs = cpool.tile([128, 1], f32, name="rs")
            nc.vector.reduce_sum(out=rs, in_=t1, axis=mybir.AxisListType.X)
            junk = cpool.tile([128, N], f32, name="junk")
            nc.vector.tensor_tensor(out=junk, in0=t1, in1=mask, op=ALU.mult)
            t0 = cpool.tile([128, 1], f32, name="t0")
            nc.vector.reduce_sum(out=t0, in_=junk, axis=mybir.AxisListType.X)
            lse = cpool.tile([128, 1], f32, name="lse")
            # row logsumexp = log(512 + sum_j e^s)
            nc.scalar.activation(
                out=lse, in_=rsum_e, func=AF.Ln, bias=n512t, scale=1.0
            )
            nc.sync.dma_start(out=lse_o[:, :], in_=lse)
            nc.sync.dma_start(out=rs_o[:, :], in_=rs)
            nc.sync.dma_start(out=t0_o[:, :], in_=t0)

    nc.compile()
    return nc


def _make_in_maps(x, y, W1, b1, W2):
    f16 = np.float16
    xTp = np.zeros((KP, N), f16)
    xTp[:D, :] = x.T.astype(f16)
    w1xTp = np.zeros((KP, KP), f16)
    w1xTp[:D, :H] = W1[:, :D].T.astype(f16)
    w1yTp = np.zeros((KP, KP), f16)
    w1yTp[:D, :H] = W1[:, D:].T.astype(f16)
    w1x3rp = np.tile(w1xTp[:, 384:400], (1, 8)).copy()
    b1p = np.zeros((KP, 1), np.float32)
    b1p[:H, 0] = b1
    # w2 slot-window: w2win[k*128 + h, 15] = W2[0, k*128 + h]
    w2winp = np.zeros((HT * 128, 48), f16)
    w2winp[:, 15] = W2[0, : HT * 128].astype(f16)
    # h-tail block-diagonal: col (32*phat + 8*phat + u) gets w2[384 + h]
    # at partition 16u + h
    w23bdp = np.zeros((128, 64), f16)
    for phat in range(2):
        for u in range(8):
            for h in range(16):
                w23bdp[16 * u + h, 32 * phat + 8 * phat + u] = np.float16(
                    W2[0, 384 + h]
                )

    in_maps = []
    for c in range(NCORES):
        yTp = np.zeros((KP, NL), f16)
        yTp[:D, :] = y[c * NL : (c + 1) * NL, :].T.astype(f16)
        mask = np.zeros((128, N), np.float32)
        for i in range(NL):
            mask[_sigma(i), c * NL + i] = 1.0
        in_maps.append(
            {
                "xT": xTp, "w1xT": w1xTp, "w1yT": w1yTp, "w1x3r": w1x3rp,
                "yT": yTp, "b1c": b1p, "w2win": w2winp, "w23bd": w23bdp,
                "maskd": mask,
            }
        )
    return in_maps


def _combine(results):
    perm = np.array([_sigma(i) for i in range(NL)])
    lse_all = np.concatenate(
        [r["lse_o"][perm, 0].astype(np.float64) for r in results]
    )
    rs_all = np.concatenate(
        [r["rs_o"][perm, 0].astype(np.float64) for r in results]
    )
    t0_all = np.concatenate(
        [r["t0_o"][perm, 0].astype(np.float64) for r in results]
    )
    t0_mean = t0_all.mean()
    lower = t0_mean - (lse_all.mean() - np.log(np.float64(N)))
    upper = t0_mean - rs_all.mean() / N
    return np.float32(lower), np.float32(upper)


def kernel(x_samples, y_samples, W1, b1, W2, b2, _trace=False):
    from concourse.bass_utils import run_bass_kernel_spmd

    nc = _build_program(float(np.float32(b2[0])))
    in_maps = _make_in_maps(
        np.asarray(x_samples, np.float32),
        np.asarray(y_samples, np.float32),
        np.asarray(W1, np.float32),
        np.asarray(b1, np.float32),
        np.asarray(W2, np.float32),
    )
    res = run_bass_kernel_spmd(
        nc, in_maps, core_ids=list(range(NCORES)), trace=_trace
    )
    out = _combine(res.results)
    if _trace:
        return out, res
    return out
